# revision 1
# baseline (speedup 1.0000x reference)
"""AtomAttentionDecoder — 8-core Trainium2 kernel.

Sharding strategy (per spec hint): data-parallel over batch (2) x
sequence-parallel over the atom-window axis (4 slices of 128 windows), one
shard per NeuronCore (8 total). Each core gets a contiguous slice of 128
windows plus a halo of 8 windows (256 atoms) on each side — enough for the
receptive-field growth of 3 blocks of local attention (needs 6) — so no
inter-core communication is required. Halo regions beyond the sequence edge
are zero-padded with atom_mask=0, which reproduces the reference's
clip+valid-mask edge semantics for all kept outputs. Small weights are
replicated to every core.
"""

import numpy as np

B, NTOK, NATOM = 2, 512, 16384
C_TOKEN, C_ATOM, C_PAIR, C_S = 384, 128, 16, 384
NQ, NK, H, NB = 32, 128, 4, 3
DH = C_ATOM // H
NW = NATOM // NQ

WSLICES = 4                 # window slices per batch
KEPT_W = NW // WSLICES      # 128 kept windows per core
HALO_W = 8                  # halo windows each side
LOC_W = KEPT_W + 2 * HALO_W  # 144 local windows
KEPT_A = KEPT_W * NQ        # 4096 kept atoms
HALO_A = HALO_W * NQ        # 256
LOC_A = LOC_W * NQ          # 4608 local atoms

_jitted = None


def _build_shard_fn():
    import jax
    import jax.numpy as jnp

    def _ln(x, g=None, b=None, eps=1e-5):
        mu = jnp.mean(x, -1, keepdims=True)
        var = jnp.mean((x - mu) ** 2, -1, keepdims=True)
        xn = (x - mu) * jax.lax.rsqrt(var + eps)
        if g is not None:
            xn = xn * g
        if b is not None:
            xn = xn + b
        return xn

    def shard_fn(a, ef, plm, am, idx,
                 Wa, lnq_g, lnq_b, Wout,
                 ag_w, ag_b, ab_w, wq, bq, wk, wv, pg, pb, wpb, wg, wo,
                 sk_w, sk_b, tg_w, tg_b, tb_w, wt1, wt2, wto, tk_w, tk_b):
        # token -> atom broadcast (local slice)
        q = a @ Wa                                    # [NTOK, C_ATOM]
        q = jnp.take(q, idx, axis=0)                  # [LOC_A, C_ATOM]
        q = q + ef
        amc = am[:, None]
        q = q * amc
        s = jnp.pad(ef, ((0, 0), (0, C_S - C_ATOM)))
        sn = _ln(s)

        # Sliding-window extraction with static slices only: window w's keys
        # are atoms [w*NQ - 48, w*NQ + 80) — i.e. 4 consecutive NQ-blocks of
        # the front-padded sequence, starting at block w.
        NBLK = NK // NQ  # 4

        def windows(t):
            # t: [LOC_A, ...] -> [LOC_W, NK, ...]
            pad = [(48, 80)] + [(0, 0)] * (t.ndim - 1)
            tp = jnp.pad(t, pad)                       # [LOC_A + NK, ...]
            blk = tp.reshape((LOC_W + NBLK, NQ) + t.shape[1:])
            w = jnp.stack([blk[j:j + LOC_W] for j in range(NBLK)], axis=1)
            return w.reshape((LOC_W, NK) + t.shape[1:])

        keymask = windows(am)                          # [LOC_W, NK]

        x = q
        inv = 1.0 / np.sqrt(DH)
        for i in range(NB):
            xa = jax.nn.sigmoid(sn @ ag_w[i] + ag_b[i]) * _ln(x) + sn @ ab_w[i]
            qh = (xa @ wq[i] + bq[i]).reshape(LOC_W, NQ, H, DH)
            kh = (xa @ wk[i]).reshape(LOC_A, H, DH)
            vh = (xa @ wv[i]).reshape(LOC_A, H, DH)
            kw = windows(kh)                                   # [LOC_W, NK, H, DH]
            vw = windows(vh)
            bias = _ln(plm, pg[i], pb[i]) @ wpb[i]             # [LOC_W, NQ, NK, H]
            scores = jnp.einsum('wqhd,wkhd->wqkh', qh, kw) * inv + bias
            scores = jnp.where(keymask[:, None, :, None] > 0, scores, -1e9)
            attn = jax.nn.softmax(scores, axis=2)
            o = jnp.einsum('wqkh,wkhd->wqhd', attn, vw).reshape(LOC_A, C_ATOM)
            gate = jax.nn.sigmoid(xa @ wg[i])
            x = x + jax.nn.sigmoid(sn @ sk_w[i] + sk_b[i]) * ((gate * o) @ wo[i])
            xt = jax.nn.sigmoid(sn @ tg_w[i] + tg_b[i]) * _ln(x) + sn @ tb_w[i]
            hsw = jax.nn.silu(xt @ wt1[i]) * (xt @ wt2[i])
            x = x + jax.nn.sigmoid(sn @ tk_w[i] + tk_b[i]) * (hsw @ wto[i])

        x = x * amc
        r = _ln(x, lnq_g, lnq_b) @ Wout                        # [LOC_A, 3]
        return r[HALO_A:HALO_A + KEPT_A]

    return jax.jit(shard_fn)


def _pad_slice(arr, lo, hi):
    """arr[lo:hi] along axis 0 with zero padding outside [0, len)."""
    n = arr.shape[0]
    lo_pad = max(0, -lo)
    hi_pad = max(0, hi - n)
    core = arr[max(lo, 0):min(hi, n)]
    if lo_pad or hi_pad:
        pad = [(lo_pad, hi_pad)] + [(0, 0)] * (arr.ndim - 1)
        core = np.pad(core, pad)
    return core


def kernel(**inputs) -> np.ndarray:
    import jax

    global _jitted
    if _jitted is None:
        _jitted = _build_shard_fn()
    f = _jitted

    wnames = ['Wa', 'lnq_g', 'lnq_b', 'Wout',
              'ag_w', 'ag_b', 'ab_w', 'wq', 'bq', 'wk', 'wv', 'pg', 'pb',
              'wpb', 'wg', 'wo', 'sk_w', 'sk_b', 'tg_w', 'tg_b', 'tb_w',
              'wt1', 'wt2', 'wto', 'tk_w', 'tk_b']
    weights = [np.asarray(inputs[k], np.float32) for k in wnames]

    a = np.asarray(inputs['a'], np.float32)
    ef = np.asarray(inputs['extra_feats'], np.float32)
    plm = np.asarray(inputs['p_lm'], np.float32)
    am = np.asarray(inputs['atom_mask'], np.float32)
    idx = np.asarray(inputs['atom_to_token_idx'], np.int32)

    devs = jax.devices()[:8]
    wdev = [[jax.device_put(w, d) for w in weights] for d in devs]

    outs = []
    for c, d in enumerate(devs):
        b, ws = divmod(c, WSLICES)
        a0 = ws * KEPT_A - HALO_A
        a1 = ws * KEPT_A + KEPT_A + HALO_A
        w0 = ws * KEPT_W - HALO_W
        w1 = ws * KEPT_W + KEPT_W + HALO_W
        sh_a = jax.device_put(a[b], d)
        sh_ef = jax.device_put(_pad_slice(ef[b], a0, a1), d)
        sh_plm = jax.device_put(_pad_slice(plm[b], w0, w1), d)
        sh_am = jax.device_put(_pad_slice(am[b], a0, a1), d)
        # idx pad value 0 is harmless: padded atoms have atom_mask == 0.
        sh_idx = jax.device_put(
            np.clip(_pad_slice(idx[b], a0, a1), 0, NTOK - 1), d)
        outs.append(f(sh_a, sh_ef, sh_plm, sh_am, sh_idx, *wdev[c]))

    outs = [np.asarray(o) for o in outs]
    full = np.empty((B, NATOM, 3), np.float32)
    for c in range(8):
        b, ws = divmod(c, WSLICES)
        full[b, ws * KEPT_A:(ws + 1) * KEPT_A] = outs[c]
    return full



# revision 2
# speedup vs baseline: 1.2357x; 1.2357x over previous
"""AtomAttentionDecoder — 8-core Trainium2 kernel, v3.

Baseline shard formulation (compiles cleanly on neuronx-cc), with two
changes: single pmap over the 8 cores (one SPMD compile, one dispatch)
and bf16 inputs for all large matmuls/einsums (PE 2x rate, half traffic).
Sharding: batch(2) x window-slices(4), halo 8 windows, no collectives.
"""

import numpy as np

B, NTOK, NATOM = 2, 512, 16384
C_TOKEN, C_ATOM, C_PAIR, C_S = 384, 128, 16, 384
NQ, NK, H, NB = 32, 128, 4, 3
DH = C_ATOM // H
NW = NATOM // NQ

WSLICES = 4
KEPT_W = NW // WSLICES
HALO_W = 8
LOC_W = KEPT_W + 2 * HALO_W
KEPT_A = KEPT_W * NQ
HALO_A = HALO_W * NQ
LOC_A = LOC_W * NQ

_jitted = None


def _build_shard_fn():
    import jax
    import jax.numpy as jnp
    bf16 = jnp.bfloat16
    f32 = jnp.float32

    def _ln(x, g=None, b=None, eps=1e-5):
        mu = jnp.mean(x, -1, keepdims=True)
        var = jnp.mean((x - mu) ** 2, -1, keepdims=True)
        xn = (x - mu) * jax.lax.rsqrt(var + eps)
        if g is not None:
            xn = xn * g
        if b is not None:
            xn = xn + b
        return xn

    def shard_fn(a, ef, plm, am, idx,
                 Wa, lnq_g, lnq_b, Wout,
                 ag_w, ag_b, ab_w, wq, bq, wk, wv, pg, pb, wpb, wg, wo,
                 sk_w, sk_b, tg_w, tg_b, tb_w, wt1, wt2, wto, tk_w, tk_b):
        q = (a.astype(bf16) @ Wa.astype(bf16)).astype(f32)   # [NTOK, C_ATOM]
        q = jnp.take(q, idx, axis=0)                         # [LOC_A, C_ATOM]
        q = q + ef
        amc = am[:, None]
        q = q * amc
        s = jnp.pad(ef, ((0, 0), (0, C_S - C_ATOM)))
        sn = _ln(s)
        snb = sn.astype(bf16)

        NBLK = NK // NQ  # 4

        def windows(t):
            pad = [(48, 80)] + [(0, 0)] * (t.ndim - 1)
            tp = jnp.pad(t, pad)
            blk = tp.reshape((LOC_W + NBLK, NQ) + t.shape[1:])
            w = jnp.stack([blk[j:j + LOC_W] for j in range(NBLK)], axis=1)
            return w.reshape((LOC_W, NK) + t.shape[1:])

        keymask = windows(am)                                # [LOC_W, NK]

        def smm(w):   # sn @ w in bf16 -> f32
            return (snb @ w.astype(bf16)).astype(f32)

        x = q
        inv = 1.0 / np.sqrt(DH)
        for i in range(NB):
            xa = jax.nn.sigmoid(smm(ag_w[i]) + ag_b[i]) * _ln(x) + smm(ab_w[i])
            xab = xa.astype(bf16)
            qh = ((xab @ wq[i].astype(bf16)).astype(f32) + bq[i]) \
                .reshape(LOC_W, NQ, H, DH).astype(bf16)
            kh = (xab @ wk[i].astype(bf16)).reshape(LOC_A, H, DH)
            vh = (xab @ wv[i].astype(bf16)).reshape(LOC_A, H, DH)
            kw = windows(kh)                                 # [LOC_W, NK, H, DH]
            vw = windows(vh)
            bias = _ln(plm, pg[i], pb[i]) @ wpb[i]           # [LOC_W, NQ, NK, H]
            scores = jnp.einsum('wqhd,wkhd->wqkh', qh, kw).astype(f32) * inv + bias
            scores = jnp.where(keymask[:, None, :, None] > 0, scores, -1e9)
            attn = jax.nn.softmax(scores, axis=2).astype(bf16)
            o = jnp.einsum('wqkh,wkhd->wqhd', attn, vw).reshape(LOC_A, C_ATOM)
            gate = jax.nn.sigmoid((xab @ wg[i].astype(bf16)).astype(f32))
            go = (gate * o.astype(f32)).astype(bf16)
            x = x + jax.nn.sigmoid(smm(sk_w[i]) + sk_b[i]) * \
                (go @ wo[i].astype(bf16)).astype(f32)
            xt = jax.nn.sigmoid(smm(tg_w[i]) + tg_b[i]) * _ln(x) + smm(tb_w[i])
            xtb = xt.astype(bf16)
            h1 = (xtb @ wt1[i].astype(bf16)).astype(f32)
            h2 = (xtb @ wt2[i].astype(bf16)).astype(f32)
            hsw = (jax.nn.silu(h1) * h2).astype(bf16)
            x = x + jax.nn.sigmoid(smm(tk_w[i]) + tk_b[i]) * \
                (hsw @ wto[i].astype(bf16)).astype(f32)

        x = x * amc
        r = _ln(x, lnq_g, lnq_b) @ Wout                      # [LOC_A, 3]
        return r[HALO_A:HALO_A + KEPT_A]

    return jax.pmap(shard_fn, devices=jax.devices()[:8])


def _pad_slice(arr, lo, hi):
    n = arr.shape[0]
    lo_pad = max(0, -lo)
    hi_pad = max(0, hi - n)
    core = arr[max(lo, 0):min(hi, n)]
    if lo_pad or hi_pad:
        pad = [(lo_pad, hi_pad)] + [(0, 0)] * (arr.ndim - 1)
        core = np.pad(core, pad)
    return core


WNAMES = ['Wa', 'lnq_g', 'lnq_b', 'Wout',
          'ag_w', 'ag_b', 'ab_w', 'wq', 'bq', 'wk', 'wv', 'pg', 'pb',
          'wpb', 'wg', 'wo', 'sk_w', 'sk_b', 'tg_w', 'tg_b', 'tb_w',
          'wt1', 'wt2', 'wto', 'tk_w', 'tk_b']


def stage_args(inputs):
    """Build stacked [8, ...] pmap args from full inputs (host side)."""
    weights = [np.asarray(inputs[k], np.float32) for k in WNAMES]
    a = np.asarray(inputs['a'], np.float32)
    ef = np.asarray(inputs['extra_feats'], np.float32)
    plm = np.asarray(inputs['p_lm'], np.float32)
    am = np.asarray(inputs['atom_mask'], np.float32)
    idx = np.asarray(inputs['atom_to_token_idx'], np.int32)

    sa, sef, splm, sam, sidx = [], [], [], [], []
    for c in range(8):
        b, ws = divmod(c, WSLICES)
        a0 = ws * KEPT_A - HALO_A
        a1 = ws * KEPT_A + KEPT_A + HALO_A
        w0 = ws * KEPT_W - HALO_W
        w1 = ws * KEPT_W + KEPT_W + HALO_W
        sa.append(a[b])
        sef.append(_pad_slice(ef[b], a0, a1))
        splm.append(_pad_slice(plm[b], w0, w1))
        sam.append(_pad_slice(am[b], a0, a1))
        sidx.append(np.clip(_pad_slice(idx[b], a0, a1), 0, NTOK - 1))
    args = [np.stack(sa), np.stack(sef), np.stack(splm), np.stack(sam),
            np.stack(sidx)]
    args += [np.broadcast_to(w, (8,) + w.shape).copy() for w in weights]
    return args


def kernel(**inputs) -> np.ndarray:
    global _jitted
    if _jitted is None:
        _jitted = _build_shard_fn()
    f = _jitted

    outs = np.asarray(f(*stage_args(inputs)))   # [8, KEPT_A, 3]
    full = np.empty((B, NATOM, 3), np.float32)
    for c in range(8):
        b, ws = divmod(c, WSLICES)
        full[b, ws * KEPT_A:(ws + 1) * KEPT_A] = outs[c]
    return full
